# revision 7
# baseline (speedup 1.0000x reference)
"""MultiHeadAttention Trainium2 kernel (8-core SPMD), v3.3.

Problem: B=2, S=2048, DIM=1024, 16 heads, head_dim=64, fp32.
Sharding: core c -> (batch b = c//4, head-group g = c%4, 4 heads each).
Each core computes, for its batch and 4 heads (2 pairs of 2 heads):
    q = x Wq'^T            (Wq' = SCALE*Wq, no bias -- see bias algebra)
    k = x Wk^T             (no bias)
    v = x Wv^T             (no bias)
    S^T[k,q] = k . q       (feature-major, both heads of a pair per chunk)
    P^T = exp(S^T) scaled per-k by m[k] = exp(SCALE * bq . k[k])
    outT[d,q] = sum_k V'[k,d] P^T[k,q]   with V' = diag(m) [V | 1]
    attn^T = outT[0:64] / outT[64]       (per-q softmax denominator)
    partial[p] = attn_pair_p^T . Pj_p^T  (per-PAIR [seq,1024] projection
                                          partial, p in {0,1})
Host sums the 8 per-pair partials per batch (2 pairs x 4 cores) and adds
bv @ proj_w.T + proj_b (V-bias and proj-bias commute through softmax/proj).

Performance structure (from perfetto/HAM analysis of prior runs):
  - The softmax exp (16.8M elements/core, Scalar-engine-only at 1
    elem/cycle/lane) is the hard floor (~142us at 1024-col activations).
    Everything else is scheduled so the Scalar engine never starves.
  - Per-chunk pipeline: score pair (2 heads as adjacent matmuls with
    auto row tile_position 0/64 that co-execute in the PE array) -> one
    exp instruction covering both heads [128, 2, 512] -> 2 AV
    accumulations.
  - Per-PAIR output projection partials unlock projection matmuls as
    dense full-array filler work through the whole second half of the
    schedule, keeping the PE HAM activity monitor from throttling the
    clock to K=4/8 (the original baseline lost ~50us to that).
  - Softmax reciprocal via DVE (iterative op, [1,512]); the attn
    normalize multiply runs on the otherwise-idle GpSimd engine.  The
    last unit instead uses exp(-ln(den)) on the (then idle) Scalar
    engine and DVE multiply to shorten the kernel tail.

Bias algebra: softmax over k of SCALE*(q0+bq).(k0+bk) equals softmax of
(SCALE*q0).k0 + SCALE*bq.k0[k] -- the q0.bk and bq.bk terms are constant in k
and drop out. The per-k term is applied multiplicatively (m[k]) by scaling V
rows, and V's bias bv adds exactly bv to every attention output row.
"""

import numpy as np

import concourse.bass as bass
import concourse.mybir as mybir
import concourse.tile as tile
from concourse import bacc
from concourse import bass_utils

F32 = mybir.dt.float32
BF16 = mybir.dt.bfloat16

P = 128
DIM = 1024
S = 2048
NH = 16
DH = 64
SCALE = 1.0 / 8.0
DC = DIM // P           # 8 contraction chunks for the qkv projections
NST = S // 512          # 4 seq tiles of 512
NCH = S // P            # 16 kpos chunks of 128
FPC = 256               # features per core (4 heads * 64)
NWARM = 16              # PE warm-up matmuls


def build_attention_bass():
    nc = bacc.Bacc(
        "TRN2",
        target_bir_lowering=False,
        debug=False,
        enable_asserts=False,
        num_devices=8,
    )
    xT = nc.dram_tensor("xT", [DIM, S], BF16, kind="ExternalInput").ap()
    wqT = nc.dram_tensor("wqT", [DIM, FPC], BF16, kind="ExternalInput").ap()
    wkT = nc.dram_tensor("wkT", [DIM, FPC], BF16, kind="ExternalInput").ap()
    wvT = nc.dram_tensor("wvT", [DIM, FPC], BF16, kind="ExternalInput").ap()
    bqz = nc.dram_tensor("bqz", [P, 2, 2], BF16, kind="ExternalInput").ap()
    pjT = nc.dram_tensor("pjT", [FPC, DIM], BF16, kind="ExternalInput").ap()
    out = nc.dram_tensor("out", [2, S, DIM], BF16, kind="ExternalOutput").ap()

    with tile.TileContext(nc) as tc:
        _attention_body(tc, xT, wqT, wkT, wvT, bqz, pjT, out)
    nc.compile()
    return nc


def _attention_body(tc, xT, wqT, wkT, wvT, bqz, pjT, out):
    nc = tc.nc
    Exp = mybir.ActivationFunctionType.Exp
    Ln = mybir.ActivationFunctionType.Ln
    Mult = mybir.AluOpType.mult

    with (
        tc.tile_pool(name="const", bufs=1) as cpool,
        tc.tile_pool(name="work", bufs=1) as wpool,
        tc.tile_pool(name="exp", bufs=4) as epool,
        tc.tile_pool(name="stage", bufs=3) as spool,
        tc.tile_pool(name="st", bufs=2, space="PSUM") as stpool,
        tc.tile_pool(name="mm", bufs=2, space="PSUM") as mmpool,
        tc.tile_pool(name="av", bufs=2, space="PSUM") as avpool,
    ):
        # ---- input loads (order = availability priority) -----------------
        wq_sb = cpool.tile([P, DC, FPC], BF16)
        wqT_r = wqT.rearrange("(dc p) f -> p dc f", p=P)
        nc.sync.dma_start(wq_sb[:, 0:2, :], wqT_r[:, 0:2, :])
        xt = cpool.tile([P, DC, S], BF16)
        xT_r = xT.rearrange("(dc p) s -> p dc s", p=P)
        nc.sync.dma_start(xt[:, 0:2, 0:512], xT_r[:, 0:2, 0:512])
        nc.sync.dma_start(wq_sb[:, 2:DC, :], wqT_r[:, 2:DC, :])
        nc.sync.dma_start(xt[:, 2:DC, 0:512], xT_r[:, 2:DC, 0:512])
        wk_sb = cpool.tile([P, DC, FPC], BF16)
        nc.sync.dma_start(wk_sb, wkT.rearrange("(dc p) f -> p dc f", p=P))
        for st in range(1, NST):
            sl = slice(512 * st, 512 * (st + 1))
            nc.sync.dma_start(xt[:, :, sl], xT_r[:, :, sl])
        wv_sb = cpool.tile([P, DC, FPC], BF16)
        nc.sync.dma_start(wv_sb, wvT.rearrange("(dc p) f -> p dc f", p=P))
        bq_sb = cpool.tile([P, 2, 2], BF16)
        nc.sync.dma_start(bq_sb, bqz)
        pj_sb = cpool.tile([P, 2, DIM], BF16)
        nc.sync.dma_start(pj_sb, pjT.rearrange("(c p) o -> p c o", p=P))

        q_sb = wpool.tile([P, 2, S], BF16)    # [dh-in-pair, pair, seq]
        k_sb = wpool.tile([P, 2, S], BF16)
        v_sb = wpool.tile([P, NCH, 4, DH + 1], BF16)
        m_sb = wpool.tile([P, NCH, 4], F32)   # exp(c) per (kpos, chunk, head)
        at_sb = wpool.tile([P, 2, S], BF16)   # normalized attn^T

        # ---- PE warm-up during the DMA lead-in ---------------------------
        warm = wpool.tile([P, 512], BF16)
        nc.vector.memset(warm, 1.0)
        wps = mmpool.tile([P, 512], F32, tag="mm")
        for i in range(NWARM):
            nc.tensor.matmul(wps, lhsT=warm[:, 0:P], rhs=warm,
                             start=True, stop=True)

        def qk_tile(p, wsb, dest, st):
            ps = mmpool.tile([P, 512], F32, tag="mm")
            for dc in range(DC):
                nc.tensor.matmul(
                    ps,
                    lhsT=wsb[:, dc, P * p:P * (p + 1)],
                    rhs=xt[:, dc, 512 * st:512 * (st + 1)],
                    start=(dc == 0),
                    stop=(dc == DC - 1),
                )
            nc.vector.tensor_copy(dest[:, p, 512 * st:512 * (st + 1)], ps)

        def c_and_m(p, st):
            # c[k] = SCALE * bq_h . k0_h[k] via block-diagonal bq operand,
            # for the 4 kpos chunks of seq-tile st.
            c_ps = mmpool.tile([P, 8], F32, tag="mm", name=f"c_ps_{p}_{st}")
            for j in range(4):
                ch = 4 * st + j
                nc.tensor.matmul(
                    c_ps[:, 2 * j:2 * j + 2],
                    lhsT=k_sb[:, p, P * ch:P * (ch + 1)],
                    rhs=bq_sb[:, p, :],
                    start=True,
                    stop=True,
                )
            for h in (0, 1):
                hh = 2 * p + h
                csl = slice(4 * st, 4 * st + 4)
                nc.scalar.activation(
                    m_sb[:, csl, hh],
                    c_ps.rearrange("p (j h) -> p j h", h=2)[:, :, h],
                    Exp,
                )
                # denominator column of V' is exp(c) itself
                nc.vector.tensor_copy(v_sb[:, csl, hh, DH], m_sb[:, csl, hh])

        def v_chunk(ch):
            ps = mmpool.tile([P, 512], F32, tag="mm")
            for dc in range(DC):
                nc.tensor.matmul(
                    ps[:, 0:FPC],
                    lhsT=xt[:, dc, P * ch:P * (ch + 1)],
                    rhs=wv_sb[:, dc, :],
                    start=(dc == 0),
                    stop=(dc == DC - 1),
                )
            nc.vector.tensor_copy(
                v_sb[:, ch, :, 0:DH],
                ps[:, 0:FPC].rearrange("p (h d) -> p h d", h=4),
            )
            scale_v(0, ch)

        def scale_v(p, ch):
            nc.vector.tensor_tensor(
                v_sb[:, ch, 2 * p:2 * p + 2, 0:DH],
                v_sb[:, ch, 2 * p:2 * p + 2, 0:DH],
                m_sb[:, ch, 2 * p:2 * p + 2, None].to_broadcast([P, 2, DH]),
                Mult,
            )

        def proj_tile(p, sm, nt):
            ps = mmpool.tile([P, 512], F32, tag="mm")
            nc.tensor.matmul(
                ps,
                lhsT=at_sb[:, p, P * sm:P * (sm + 1)],
                rhs=pj_sb[:, p, 512 * nt:512 * (nt + 1)],
                start=True,
                stop=True,
            )
            stg = spool.tile([P, 512], BF16, tag="out", bufs=3)
            nc.vector.tensor_copy(stg, ps)
            nc.sync.dma_start(
                out[p, P * sm:P * (sm + 1), 512 * nt:512 * (nt + 1)], stg
            )

        def attention_unit(p, qt, fillers, tail=False):
            """One (pair, qtile) unit: 16 kpos chunks; per chunk a score
            pair (row-tiled heads), one exp over both heads, and the two
            AV accumulations. fillers: thunks emitted one per chunk."""
            qsl = slice(512 * qt, 512 * (qt + 1))
            pav = [avpool.tile([DH + 1, 512], F32, tag="av",
                               name=f"pav_{p}_{qt}_{h}")
                   for h in (0, 1)]
            for ch in range(NCH):
                if fillers:
                    fillers.pop(0)()
                st_t = stpool.tile([P, 2, 512], F32, tag="st",
                                   name=f"st_{p}_{qt}_{ch}")
                for h in (0, 1):
                    nc.tensor.matmul(
                        st_t[:, h, :],
                        lhsT=k_sb[DH * h:DH * (h + 1), p, P * ch:P * (ch + 1)],
                        rhs=q_sb[DH * h:DH * (h + 1), p, qsl],
                        start=True,
                        stop=True,
                    )
                e_t = epool.tile([P, 2, 512], BF16, tag="e",
                                 name=f"e_{p}_{qt}_{ch}")
                nc.scalar.activation(e_t, st_t, Exp)
                for h in (0, 1):
                    nc.tensor.matmul(
                        pav[h],
                        lhsT=v_sb[:, ch, 2 * p + h, :],
                        rhs=e_t[:, h, :],
                        start=(ch == 0),
                        stop=(ch == NCH - 1),
                    )
            while fillers:
                fillers.pop(0)()
            # ---- epilogue: normalize attn^T ------------------------------
            un = [spool.tile([DH + 1, 512], F32, tag="un", bufs=4,
                             name=f"un_{p}_{qt}_{h}")
                  for h in (0, 1)]
            for h in (0, 1):
                nc.vector.tensor_copy(un[h], pav[h])
            for h in (0, 1):
                rec = spool.tile([1, 512], F32, tag="rec", bufs=2)
                if tail:
                    # Scalar engine is idle in the tail; exp(-ln(d)) there
                    nl = spool.tile([1, 512], F32, tag="nl", bufs=2)
                    nc.scalar.activation(nl, un[h][DH:DH + 1, :], Ln)
                    nc.scalar.activation(rec, nl, Exp, scale=-1.0)
                else:
                    nc.vector.reciprocal(rec, un[h][DH:DH + 1, :])
                rb = spool.tile([DH, 512], F32, tag="rb", bufs=2)
                nc.gpsimd.partition_broadcast(rb, rec)
                eng = nc.vector if tail else nc.gpsimd
                eng.tensor_tensor(
                    at_sb[DH * h:DH * (h + 1), p, qsl],
                    un[h][0:DH, :],
                    rb,
                    Mult,
                )

        # ---- pre-attention minimum ---------------------------------------
        qk_tile(0, wq_sb, q_sb, 0)
        qk_tile(0, wk_sb, k_sb, 0)
        c_and_m(0, 0)
        for ch in range(4):
            v_chunk(ch)

        # ---- unit schedule with woven fillers ----------------------------
        attention_unit(0, 0, [
            lambda: qk_tile(0, wk_sb, k_sb, 1),
            lambda: (c_and_m(0, 1), v_chunk(4)),
            lambda: v_chunk(5),
            lambda: v_chunk(6),
            lambda: v_chunk(7),
            lambda: qk_tile(0, wk_sb, k_sb, 2),
            lambda: (c_and_m(0, 2), v_chunk(8)),
            lambda: v_chunk(9),
            lambda: v_chunk(10),
            lambda: qk_tile(0, wk_sb, k_sb, 3),
            lambda: (c_and_m(0, 3), v_chunk(11)),
            lambda: v_chunk(12),
            lambda: v_chunk(13),
            lambda: v_chunk(14),
            lambda: v_chunk(15),
            lambda: qk_tile(0, wq_sb, q_sb, 1),
        ])
        attention_unit(0, 1, [
            lambda: qk_tile(0, wq_sb, q_sb, 2),
            lambda: qk_tile(1, wk_sb, k_sb, 0),
            lambda: c_and_m(1, 0),
            lambda: qk_tile(1, wk_sb, k_sb, 1),
            lambda: c_and_m(1, 1),
        ])
        attention_unit(0, 2, [
            lambda: qk_tile(0, wq_sb, q_sb, 3),
            lambda: qk_tile(1, wk_sb, k_sb, 2),
            lambda: c_and_m(1, 2),
            lambda: qk_tile(1, wk_sb, k_sb, 3),
            lambda: c_and_m(1, 3),
        ] + [(lambda c0=c: scale_v(1, c0)) for c in range(6)] + [
            (lambda s=sm, n=nt: proj_tile(0, s, n))
            for sm in range(0, 2) for nt in range(2)
        ])
        attention_unit(0, 3, [
            lambda: qk_tile(1, wq_sb, q_sb, 0),
            lambda: qk_tile(1, wq_sb, q_sb, 1),
        ] + [(lambda c0=c: scale_v(1, c0)) for c in range(6, NCH)] + [
            (lambda s=sm, n=nt: proj_tile(0, s, n))
            for sm in range(2, 4) for nt in range(2)
        ])
        attention_unit(1, 0, [
            lambda: qk_tile(1, wq_sb, q_sb, 2),
        ] + [
            (lambda s=sm, n=nt: proj_tile(0, s, n))
            for sm in range(4, 10) for nt in range(2)
        ])
        attention_unit(1, 1, [
            lambda: qk_tile(1, wq_sb, q_sb, 3),
        ] + [
            (lambda s=sm, n=nt: proj_tile(0, s, n))
            for sm in range(10, 16) for nt in range(2)
        ])
        attention_unit(1, 2, [
            (lambda s=sm, n=nt: proj_tile(1, s, n))
            for sm in range(0, 8) for nt in range(2)
        ])
        attention_unit(1, 3, [
            (lambda s=sm, n=nt: proj_tile(1, s, n))
            for sm in range(8, 12) for nt in range(2)
        ], tail=True)
        for sm in range(12, 16):
            for nt in range(2):
                proj_tile(1, sm, nt)


# ----------------------------------------------------------------------------
# host-side wrapper
# ----------------------------------------------------------------------------

_NC_CACHE = {}


def _get_nc():
    if "nc" not in _NC_CACHE:
        _NC_CACHE["nc"] = build_attention_bass()
    return _NC_CACHE["nc"]


def make_in_maps(x, qkv_w, qkv_b, proj_w):
    """Build the 8 per-core input dicts (host-side sharding)."""
    import ml_dtypes

    bf16 = ml_dtypes.bfloat16
    in_maps = []
    for c in range(8):
        b, g = divmod(c, 4)
        fsl = slice(g * FPC, (g + 1) * FPC)
        wq = (SCALE * qkv_w[0 * DIM:1 * DIM][fsl]).T     # (1024, 256)
        wk = qkv_w[1 * DIM:2 * DIM][fsl].T
        wv = qkv_w[2 * DIM:3 * DIM][fsl].T
        bq = SCALE * qkv_b[0 * DIM:1 * DIM][fsl]         # (256,)
        bqz = np.zeros((P, 2, 2), np.float32)
        for p in range(2):
            for h in range(2):
                bqz[DH * h:DH * (h + 1), p, h] = bq[(2 * p + h) * DH:(2 * p + h + 1) * DH]
        pj = proj_w[:, fsl].T                            # (256, 1024)
        in_maps.append({
            "xT": np.ascontiguousarray(x[b].T).astype(bf16),
            "wqT": np.ascontiguousarray(wq).astype(bf16),
            "wkT": np.ascontiguousarray(wk).astype(bf16),
            "wvT": np.ascontiguousarray(wv).astype(bf16),
            "bqz": bqz.astype(bf16),
            "pjT": np.ascontiguousarray(pj).astype(bf16),
        })
    return in_maps


def combine_outputs(results, qkv_b, proj_w, proj_b):
    """Sum per-pair partials and add the host-folded biases."""
    bv = qkv_b[2 * DIM:3 * DIM]
    host_bias = bv @ proj_w.T + proj_b                   # (1024,)
    out = np.empty((2, S, DIM), np.float32)
    for b in range(2):
        acc = np.zeros((S, DIM), np.float32)
        for g in range(4):
            r = np.asarray(results[4 * b + g]["out"], np.float32)
            acc += r[0]
            acc += r[1]
        out[b] = acc + host_bias[None, :]
    return out


def kernel(x, qkv_w, qkv_b, proj_w, proj_b):
    x = np.asarray(x, np.float32)
    qkv_w = np.asarray(qkv_w, np.float32)
    qkv_b = np.asarray(qkv_b, np.float32)
    proj_w = np.asarray(proj_w, np.float32)
    proj_b = np.asarray(proj_b, np.float32)

    nc = _get_nc()
    in_maps = make_in_maps(x, qkv_w, qkv_b, proj_w)
    res = bass_utils.run_bass_kernel_spmd(nc, in_maps, core_ids=list(range(8)))
    return combine_outputs(res.results, qkv_b, proj_w, proj_b)


# revision 8
# speedup vs baseline: 1.2128x; 1.2128x over previous
"""MultiHeadAttention Trainium2 kernel (8-core SPMD), v3.3.

Problem: B=2, S=2048, DIM=1024, 16 heads, head_dim=64, fp32.
Sharding: core c -> (batch b = c//4, head-group g = c%4, 4 heads each).
Each core computes, for its batch and 4 heads (2 pairs of 2 heads):
    q = x Wq'^T            (Wq' = SCALE*Wq, no bias -- see bias algebra)
    k = x Wk^T             (no bias)
    v = x Wv^T             (no bias)
    S^T[k,q] = k . q       (feature-major, both heads of a pair per chunk)
    P^T = exp(S^T) scaled per-k by m[k] = exp(SCALE * bq . k[k])
    outT[d,q] = sum_k V'[k,d] P^T[k,q]   with V' = diag(m) [V | 1]
    attn^T = outT[0:64] / outT[64]       (per-q softmax denominator)
    partial[p] = attn_pair_p^T . Pj_p^T  (per-PAIR [seq,1024] projection
                                          partial, p in {0,1})
Host sums the 8 per-pair partials per batch (2 pairs x 4 cores) and adds
bv @ proj_w.T + proj_b (V-bias and proj-bias commute through softmax/proj).

Performance structure (from perfetto/HAM analysis of prior runs):
  - The softmax exp (16.8M elements/core, Scalar-engine-only at 1
    elem/cycle/lane) is the hard floor (~142us at 1024-col activations).
    Everything else is scheduled so the Scalar engine never starves.
  - Per-chunk pipeline: score pair (2 heads as adjacent matmuls with
    auto row tile_position 0/64 that co-execute in the PE array) -> one
    exp instruction covering both heads [128, 2, 512] -> 2 AV
    accumulations.
  - Per-PAIR output projection partials unlock projection matmuls as
    dense full-array filler work through the whole second half of the
    schedule, keeping the PE HAM activity monitor from throttling the
    clock to K=4/8 (the original baseline lost ~50us to that).
  - Softmax reciprocal via DVE (iterative op, [1,512]); the attn
    normalize multiply runs on the otherwise-idle GpSimd engine.  The
    last unit instead uses exp(-ln(den)) on the (then idle) Scalar
    engine and DVE multiply to shorten the kernel tail.

Bias algebra: softmax over k of SCALE*(q0+bq).(k0+bk) equals softmax of
(SCALE*q0).k0 + SCALE*bq.k0[k] -- the q0.bk and bq.bk terms are constant in k
and drop out. The per-k term is applied multiplicatively (m[k]) by scaling V
rows, and V's bias bv adds exactly bv to every attention output row.
"""

import numpy as np

import concourse.bass as bass
import concourse.mybir as mybir
import concourse.tile as tile
from concourse import bacc
from concourse import bass_utils

F32 = mybir.dt.float32
BF16 = mybir.dt.bfloat16

P = 128
DIM = 1024
S = 2048
NH = 16
DH = 64
SCALE = 1.0 / 8.0
DC = DIM // P           # 8 contraction chunks for the qkv projections
NST = S // 512          # 4 seq tiles of 512
NCH = S // P            # 16 kpos chunks of 128
FPC = 256               # features per core (4 heads * 64)
NWARM = 16              # PE warm-up matmuls


def build_attention_bass():
    nc = bacc.Bacc(
        "TRN2",
        target_bir_lowering=False,
        debug=False,
        enable_asserts=False,
        num_devices=8,
    )
    xT = nc.dram_tensor("xT", [DIM, S], BF16, kind="ExternalInput").ap()
    wqT = nc.dram_tensor("wqT", [DIM, FPC], BF16, kind="ExternalInput").ap()
    wkT = nc.dram_tensor("wkT", [DIM, FPC], BF16, kind="ExternalInput").ap()
    wvT = nc.dram_tensor("wvT", [DIM, FPC], BF16, kind="ExternalInput").ap()
    bqz = nc.dram_tensor("bqz", [P, 2, 2], BF16, kind="ExternalInput").ap()
    pjT = nc.dram_tensor("pjT", [FPC, DIM], BF16, kind="ExternalInput").ap()
    out = nc.dram_tensor("out", [2, S, DIM], BF16, kind="ExternalOutput").ap()

    with tile.TileContext(nc) as tc:
        _attention_body(tc, xT, wqT, wkT, wvT, bqz, pjT, out)
    nc.compile()
    return nc


def _attention_body(tc, xT, wqT, wkT, wvT, bqz, pjT, out):
    nc = tc.nc
    Exp = mybir.ActivationFunctionType.Exp
    Ln = mybir.ActivationFunctionType.Ln
    Mult = mybir.AluOpType.mult

    with (
        tc.tile_pool(name="const", bufs=1) as cpool,
        tc.tile_pool(name="work", bufs=1) as wpool,
        tc.tile_pool(name="exp", bufs=4) as epool,
        tc.tile_pool(name="stage", bufs=3) as spool,
        tc.tile_pool(name="st", bufs=2, space="PSUM") as stpool,
        tc.tile_pool(name="mm", bufs=2, space="PSUM") as mmpool,
        tc.tile_pool(name="av", bufs=2, space="PSUM") as avpool,
    ):
        # ---- input loads (order = availability priority) -----------------
        wq_sb = cpool.tile([P, DC, FPC], BF16)
        wqT_r = wqT.rearrange("(dc p) f -> p dc f", p=P)
        nc.sync.dma_start(wq_sb[:, 0:2, :], wqT_r[:, 0:2, :])
        xt = cpool.tile([P, DC, S], BF16)
        xT_r = xT.rearrange("(dc p) s -> p dc s", p=P)
        nc.sync.dma_start(xt[:, 0:2, 0:512], xT_r[:, 0:2, 0:512])
        nc.sync.dma_start(wq_sb[:, 2:DC, :], wqT_r[:, 2:DC, :])
        nc.sync.dma_start(xt[:, 2:DC, 0:512], xT_r[:, 2:DC, 0:512])
        wk_sb = cpool.tile([P, DC, FPC], BF16)
        nc.sync.dma_start(wk_sb, wkT.rearrange("(dc p) f -> p dc f", p=P))
        for st in range(1, NST):
            sl = slice(512 * st, 512 * (st + 1))
            nc.sync.dma_start(xt[:, :, sl], xT_r[:, :, sl])
        wv_sb = cpool.tile([P, DC, FPC], BF16)
        nc.sync.dma_start(wv_sb, wvT.rearrange("(dc p) f -> p dc f", p=P))
        bq_sb = cpool.tile([P, 2, 2], BF16)
        nc.sync.dma_start(bq_sb, bqz)
        pj_sb = cpool.tile([P, 2, DIM], BF16)
        nc.sync.dma_start(pj_sb, pjT.rearrange("(c p) o -> p c o", p=P))

        q_sb = wpool.tile([P, 2, S], BF16)    # [dh-in-pair, pair, seq]
        k_sb = wpool.tile([P, 2, S], BF16)
        v_sb = wpool.tile([P, NCH, 4, DH + 1], BF16)
        m_sb = wpool.tile([P, NCH, 4], F32)   # exp(c) per (kpos, chunk, head)
        at_sb = wpool.tile([P, 2, S], BF16)   # normalized attn^T

        # ---- PE warm-up during the DMA lead-in ---------------------------
        warm = wpool.tile([P, 512], BF16)
        nc.vector.memset(warm, 1.0)
        wps = mmpool.tile([P, 512], F32, tag="mm")
        for i in range(NWARM):
            nc.tensor.matmul(wps, lhsT=warm[:, 0:P], rhs=warm,
                             start=True, stop=True)

        def qk_tile(p, wsb, dest, st):
            ps = mmpool.tile([P, 512], F32, tag="mm")
            for dc in range(DC):
                nc.tensor.matmul(
                    ps,
                    lhsT=wsb[:, dc, P * p:P * (p + 1)],
                    rhs=xt[:, dc, 512 * st:512 * (st + 1)],
                    start=(dc == 0),
                    stop=(dc == DC - 1),
                )
            nc.vector.tensor_copy(dest[:, p, 512 * st:512 * (st + 1)], ps)

        def c_and_m(p, st):
            # c[k] = SCALE * bq_h . k0_h[k] via block-diagonal bq operand,
            # for the 4 kpos chunks of seq-tile st.
            c_ps = mmpool.tile([P, 8], F32, tag="mm", name=f"c_ps_{p}_{st}")
            for j in range(4):
                ch = 4 * st + j
                nc.tensor.matmul(
                    c_ps[:, 2 * j:2 * j + 2],
                    lhsT=k_sb[:, p, P * ch:P * (ch + 1)],
                    rhs=bq_sb[:, p, :],
                    start=True,
                    stop=True,
                )
            for h in (0, 1):
                hh = 2 * p + h
                csl = slice(4 * st, 4 * st + 4)
                nc.scalar.activation(
                    m_sb[:, csl, hh],
                    c_ps.rearrange("p (j h) -> p j h", h=2)[:, :, h],
                    Exp,
                )
                # denominator column of V' is exp(c) itself
                nc.vector.tensor_copy(v_sb[:, csl, hh, DH], m_sb[:, csl, hh])

        def v_chunk(ch):
            ps = mmpool.tile([P, 512], F32, tag="mm")
            for dc in range(DC):
                nc.tensor.matmul(
                    ps[:, 0:FPC],
                    lhsT=xt[:, dc, P * ch:P * (ch + 1)],
                    rhs=wv_sb[:, dc, :],
                    start=(dc == 0),
                    stop=(dc == DC - 1),
                )
            nc.vector.tensor_copy(
                v_sb[:, ch, :, 0:DH],
                ps[:, 0:FPC].rearrange("p (h d) -> p h d", h=4),
            )
            scale_v(0, ch)

        def scale_v(p, ch):
            nc.vector.tensor_tensor(
                v_sb[:, ch, 2 * p:2 * p + 2, 0:DH],
                v_sb[:, ch, 2 * p:2 * p + 2, 0:DH],
                m_sb[:, ch, 2 * p:2 * p + 2, None].to_broadcast([P, 2, DH]),
                Mult,
            )

        def proj_tile(p, sm, nt):
            ps = mmpool.tile([P, 512], F32, tag="mm")
            nc.tensor.matmul(
                ps,
                lhsT=at_sb[:, p, P * sm:P * (sm + 1)],
                rhs=pj_sb[:, p, 512 * nt:512 * (nt + 1)],
                start=True,
                stop=True,
            )
            stg = spool.tile([P, 512], BF16, tag="out", bufs=3)
            nc.vector.tensor_copy(stg, ps)
            nc.sync.dma_start(
                out[p, P * sm:P * (sm + 1), 512 * nt:512 * (nt + 1)], stg
            )

        def attention_unit(p, qt, fillers, tail=False):
            """One (pair, qtile) unit: 16 kpos chunks; per chunk a score
            pair (row-tiled heads), one exp over both heads, and the two
            AV accumulations. fillers: thunks emitted one per chunk."""
            qsl = slice(512 * qt, 512 * (qt + 1))
            pav = [avpool.tile([DH + 1, 512], F32, tag="av",
                               name=f"pav_{p}_{qt}_{h}")
                   for h in (0, 1)]
            for ch in range(NCH):
                if fillers:
                    fillers.pop(0)()
                st_t = stpool.tile([P, 2, 512], F32, tag="st",
                                   name=f"st_{p}_{qt}_{ch}")
                for h in (0, 1):
                    nc.tensor.matmul(
                        st_t[:, h, :],
                        lhsT=k_sb[DH * h:DH * (h + 1), p, P * ch:P * (ch + 1)],
                        rhs=q_sb[DH * h:DH * (h + 1), p, qsl],
                        start=True,
                        stop=True,
                    )
                e_t = epool.tile([P, 2, 512], BF16, tag="e",
                                 name=f"e_{p}_{qt}_{ch}")
                nc.scalar.activation(e_t, st_t, Exp)
                for h in (0, 1):
                    nc.tensor.matmul(
                        pav[h],
                        lhsT=v_sb[:, ch, 2 * p + h, :],
                        rhs=e_t[:, h, :],
                        start=(ch == 0),
                        stop=(ch == NCH - 1),
                    )
            while fillers:
                fillers.pop(0)()
            # ---- epilogue: normalize attn^T ------------------------------
            un = [spool.tile([DH + 1, 512], F32, tag="un", bufs=4,
                             name=f"un_{p}_{qt}_{h}")
                  for h in (0, 1)]
            for h in (0, 1):
                nc.vector.tensor_copy(un[h], pav[h])
            for h in (0, 1):
                rec = spool.tile([1, 512], F32, tag="rec", bufs=2)
                if tail:
                    # Scalar engine is idle in the tail; exp(-ln(d)) there
                    nl = spool.tile([1, 512], F32, tag="nl", bufs=2)
                    nc.scalar.activation(nl, un[h][DH:DH + 1, :], Ln)
                    nc.scalar.activation(rec, nl, Exp, scale=-1.0)
                else:
                    nc.vector.reciprocal(rec, un[h][DH:DH + 1, :])
                rb = spool.tile([DH, 512], F32, tag="rb", bufs=2)
                nc.gpsimd.partition_broadcast(rb, rec)
                nc.vector.tensor_tensor(
                    at_sb[DH * h:DH * (h + 1), p, qsl],
                    un[h][0:DH, :],
                    rb,
                    Mult,
                )

        # ---- pre-attention minimum ---------------------------------------
        qk_tile(0, wq_sb, q_sb, 0)
        qk_tile(0, wk_sb, k_sb, 0)
        c_and_m(0, 0)
        for ch in range(4):
            v_chunk(ch)

        # ---- unit schedule with woven fillers ----------------------------
        attention_unit(0, 0, [
            lambda: qk_tile(0, wk_sb, k_sb, 1),
            lambda: (c_and_m(0, 1), v_chunk(4)),
            lambda: v_chunk(5),
            lambda: v_chunk(6),
            lambda: v_chunk(7),
            lambda: qk_tile(0, wk_sb, k_sb, 2),
            lambda: (c_and_m(0, 2), v_chunk(8)),
            lambda: v_chunk(9),
            lambda: v_chunk(10),
            lambda: qk_tile(0, wk_sb, k_sb, 3),
            lambda: (c_and_m(0, 3), v_chunk(11)),
            lambda: v_chunk(12),
            lambda: v_chunk(13),
            lambda: v_chunk(14),
            lambda: v_chunk(15),
            lambda: qk_tile(0, wq_sb, q_sb, 1),
        ])
        attention_unit(0, 1, [
            lambda: qk_tile(0, wq_sb, q_sb, 2),
            lambda: qk_tile(1, wk_sb, k_sb, 0),
            lambda: c_and_m(1, 0),
            lambda: qk_tile(1, wk_sb, k_sb, 1),
            lambda: c_and_m(1, 1),
        ])
        attention_unit(0, 2, [
            lambda: qk_tile(0, wq_sb, q_sb, 3),
            lambda: qk_tile(1, wk_sb, k_sb, 2),
            lambda: c_and_m(1, 2),
            lambda: qk_tile(1, wk_sb, k_sb, 3),
            lambda: c_and_m(1, 3),
        ] + [(lambda c0=c: scale_v(1, c0)) for c in range(6)] + [
            (lambda s=sm, n=nt: proj_tile(0, s, n))
            for sm in range(0, 2) for nt in range(2)
        ])
        attention_unit(0, 3, [
            lambda: qk_tile(1, wq_sb, q_sb, 0),
            lambda: qk_tile(1, wq_sb, q_sb, 1),
        ] + [(lambda c0=c: scale_v(1, c0)) for c in range(6, NCH)] + [
            (lambda s=sm, n=nt: proj_tile(0, s, n))
            for sm in range(2, 4) for nt in range(2)
        ])
        attention_unit(1, 0, [
            lambda: qk_tile(1, wq_sb, q_sb, 2),
        ] + [
            (lambda s=sm, n=nt: proj_tile(0, s, n))
            for sm in range(4, 10) for nt in range(2)
        ])
        attention_unit(1, 1, [
            lambda: qk_tile(1, wq_sb, q_sb, 3),
        ] + [
            (lambda s=sm, n=nt: proj_tile(0, s, n))
            for sm in range(10, 16) for nt in range(2)
        ])
        attention_unit(1, 2, [
            (lambda s=sm, n=nt: proj_tile(1, s, n))
            for sm in range(0, 8) for nt in range(2)
        ])
        attention_unit(1, 3, [
            (lambda s=sm, n=nt: proj_tile(1, s, n))
            for sm in range(8, 12) for nt in range(2)
        ], tail=True)
        for sm in range(12, 16):
            for nt in range(2):
                proj_tile(1, sm, nt)


# ----------------------------------------------------------------------------
# host-side wrapper
# ----------------------------------------------------------------------------

_NC_CACHE = {}


def _get_nc():
    if "nc" not in _NC_CACHE:
        _NC_CACHE["nc"] = build_attention_bass()
    return _NC_CACHE["nc"]


def make_in_maps(x, qkv_w, qkv_b, proj_w):
    """Build the 8 per-core input dicts (host-side sharding)."""
    import ml_dtypes

    bf16 = ml_dtypes.bfloat16
    in_maps = []
    for c in range(8):
        b, g = divmod(c, 4)
        fsl = slice(g * FPC, (g + 1) * FPC)
        wq = (SCALE * qkv_w[0 * DIM:1 * DIM][fsl]).T     # (1024, 256)
        wk = qkv_w[1 * DIM:2 * DIM][fsl].T
        wv = qkv_w[2 * DIM:3 * DIM][fsl].T
        bq = SCALE * qkv_b[0 * DIM:1 * DIM][fsl]         # (256,)
        bqz = np.zeros((P, 2, 2), np.float32)
        for p in range(2):
            for h in range(2):
                bqz[DH * h:DH * (h + 1), p, h] = bq[(2 * p + h) * DH:(2 * p + h + 1) * DH]
        pj = proj_w[:, fsl].T                            # (256, 1024)
        in_maps.append({
            "xT": np.ascontiguousarray(x[b].T).astype(bf16),
            "wqT": np.ascontiguousarray(wq).astype(bf16),
            "wkT": np.ascontiguousarray(wk).astype(bf16),
            "wvT": np.ascontiguousarray(wv).astype(bf16),
            "bqz": bqz.astype(bf16),
            "pjT": np.ascontiguousarray(pj).astype(bf16),
        })
    return in_maps


def combine_outputs(results, qkv_b, proj_w, proj_b):
    """Sum per-pair partials and add the host-folded biases."""
    bv = qkv_b[2 * DIM:3 * DIM]
    host_bias = bv @ proj_w.T + proj_b                   # (1024,)
    out = np.empty((2, S, DIM), np.float32)
    for b in range(2):
        acc = np.zeros((S, DIM), np.float32)
        for g in range(4):
            r = np.asarray(results[4 * b + g]["out"], np.float32)
            acc += r[0]
            acc += r[1]
        out[b] = acc + host_bias[None, :]
    return out


def kernel(x, qkv_w, qkv_b, proj_w, proj_b):
    x = np.asarray(x, np.float32)
    qkv_w = np.asarray(qkv_w, np.float32)
    qkv_b = np.asarray(qkv_b, np.float32)
    proj_w = np.asarray(proj_w, np.float32)
    proj_b = np.asarray(proj_b, np.float32)

    nc = _get_nc()
    in_maps = make_in_maps(x, qkv_w, qkv_b, proj_w)
    res = bass_utils.run_bass_kernel_spmd(nc, in_maps, core_ids=list(range(8)))
    return combine_outputs(res.results, qkv_b, proj_w, proj_b)


# revision 13
# speedup vs baseline: 1.2370x; 1.0200x over previous
"""MultiHeadAttention Trainium2 kernel (8-core SPMD), v3.3.

Problem: B=2, S=2048, DIM=1024, 16 heads, head_dim=64, fp32.
Sharding: core c -> (batch b = c//4, head-group g = c%4, 4 heads each).
Each core computes, for its batch and 4 heads (2 pairs of 2 heads):
    q = x Wq'^T            (Wq' = SCALE*Wq, no bias -- see bias algebra)
    k = x Wk^T             (no bias)
    v = x Wv^T             (no bias)
    S^T[k,q] = k . q       (feature-major, both heads of a pair per chunk)
    P^T = exp(S^T) scaled per-k by m[k] = exp(SCALE * bq . k[k])
    outT[d,q] = sum_k V'[k,d] P^T[k,q]   with V' = diag(m) [V | 1]
    attn^T = outT[0:64] / outT[64]       (per-q softmax denominator)
    partial[p] = attn_pair_p^T . Pj_p^T  (per-PAIR [seq,1024] projection
                                          partial, p in {0,1})
Host sums the 8 per-pair partials per batch (2 pairs x 4 cores) and adds
bv @ proj_w.T + proj_b (V-bias and proj-bias commute through softmax/proj).

Performance structure (from perfetto/HAM analysis of prior runs):
  - The softmax exp (16.8M elements/core, Scalar-engine-only at 1
    elem/cycle/lane) is the hard floor (~142us at 1024-col activations).
    Everything else is scheduled so the Scalar engine never starves.
  - Per-chunk pipeline: score pair (2 heads as adjacent matmuls with
    auto row tile_position 0/64 that co-execute in the PE array) -> one
    exp instruction covering both heads [128, 2, 512] -> 2 AV
    accumulations.
  - Per-PAIR output projection partials unlock projection matmuls as
    dense full-array filler work through the whole second half of the
    schedule, keeping the PE HAM activity monitor from throttling the
    clock to K=4/8 (the original baseline lost ~50us to that).
  - Softmax reciprocal via DVE (iterative op, [1,512]); the attn
    normalize multiply runs on the otherwise-idle GpSimd engine.  The
    last unit instead uses exp(-ln(den)) on the (then idle) Scalar
    engine and DVE multiply to shorten the kernel tail.

Bias algebra: softmax over k of SCALE*(q0+bq).(k0+bk) equals softmax of
(SCALE*q0).k0 + SCALE*bq.k0[k] -- the q0.bk and bq.bk terms are constant in k
and drop out. The per-k term is applied multiplicatively (m[k]) by scaling V
rows, and V's bias bv adds exactly bv to every attention output row.
"""

import numpy as np

import concourse.bass as bass
import concourse.mybir as mybir
import concourse.tile as tile
from concourse import bacc
from concourse import bass_utils

F32 = mybir.dt.float32
BF16 = mybir.dt.bfloat16

P = 128
DIM = 1024
S = 2048
NH = 16
DH = 64
SCALE = 1.0 / 8.0
DC = DIM // P           # 8 contraction chunks for the qkv projections
NST = S // 512          # 4 seq tiles of 512
NCH = S // P            # 16 kpos chunks of 128
FPC = 256               # features per core (4 heads * 64)
NWARM = 16              # PE warm-up matmuls


def build_attention_bass():
    nc = bacc.Bacc(
        "TRN2",
        target_bir_lowering=False,
        debug=False,
        enable_asserts=False,
        num_devices=8,
    )
    xT = nc.dram_tensor("xT", [DIM, S], BF16, kind="ExternalInput").ap()
    wqT = nc.dram_tensor("wqT", [DIM, FPC], BF16, kind="ExternalInput").ap()
    wkT = nc.dram_tensor("wkT", [DIM, FPC], BF16, kind="ExternalInput").ap()
    wvT = nc.dram_tensor("wvT", [DIM, FPC], BF16, kind="ExternalInput").ap()
    bqz = nc.dram_tensor("bqz", [P, 2, 2], BF16, kind="ExternalInput").ap()
    pjT = nc.dram_tensor("pjT", [FPC, DIM], BF16, kind="ExternalInput").ap()
    out = nc.dram_tensor("out", [S, DIM], BF16, kind="ExternalOutput").ap()

    with tile.TileContext(nc) as tc:
        _attention_body(tc, xT, wqT, wkT, wvT, bqz, pjT, out)
    nc.compile()
    return nc


def _attention_body(tc, xT, wqT, wkT, wvT, bqz, pjT, out):
    nc = tc.nc
    Exp = mybir.ActivationFunctionType.Exp
    Ln = mybir.ActivationFunctionType.Ln
    Mult = mybir.AluOpType.mult

    with (
        tc.tile_pool(name="const", bufs=1) as cpool,
        tc.tile_pool(name="work", bufs=1) as wpool,
        tc.tile_pool(name="exp", bufs=4) as epool,
        tc.tile_pool(name="stage", bufs=3) as spool,
        tc.tile_pool(name="st", bufs=2, space="PSUM") as stpool,
        tc.tile_pool(name="mm", bufs=2, space="PSUM") as mmpool,
        tc.tile_pool(name="av", bufs=2, space="PSUM") as avpool,
    ):
        # ---- input loads (order = availability priority) -----------------
        wq_sb = cpool.tile([P, DC, FPC], BF16)
        wqT_r = wqT.rearrange("(dc p) f -> p dc f", p=P)
        nc.sync.dma_start(wq_sb[:, 0:2, :], wqT_r[:, 0:2, :])
        xt = cpool.tile([P, DC, S], BF16)
        xT_r = xT.rearrange("(dc p) s -> p dc s", p=P)
        nc.sync.dma_start(xt[:, 0:2, 0:512], xT_r[:, 0:2, 0:512])
        nc.sync.dma_start(wq_sb[:, 2:DC, :], wqT_r[:, 2:DC, :])
        nc.sync.dma_start(xt[:, 2:DC, 0:512], xT_r[:, 2:DC, 0:512])
        wk_sb = cpool.tile([P, DC, FPC], BF16)
        nc.sync.dma_start(wk_sb, wkT.rearrange("(dc p) f -> p dc f", p=P))
        for st in range(1, NST):
            sl = slice(512 * st, 512 * (st + 1))
            nc.sync.dma_start(xt[:, :, sl], xT_r[:, :, sl])
        wv_sb = cpool.tile([P, DC, FPC], BF16)
        nc.sync.dma_start(wv_sb, wvT.rearrange("(dc p) f -> p dc f", p=P))
        bq_sb = cpool.tile([P, 2, 2], BF16)
        nc.sync.dma_start(bq_sb, bqz)
        pj_sb = cpool.tile([P, 2, DIM], BF16)
        nc.sync.dma_start(pj_sb, pjT.rearrange("(c p) o -> p c o", p=P))

        ones = wpool.tile([DH + 1, DH], F32)  # bcast operand (row 64 used)
        nc.vector.memset(ones, 1.0)
        q_sb = wpool.tile([P, 2, S], BF16)    # [dh-in-pair, pair, seq]
        k_sb = wpool.tile([P, 2, S], BF16)
        v_sb = wpool.tile([P, NCH, 4, DH + 1], BF16)
        m_sb = wpool.tile([P, NCH, 4], F32)   # exp(c) per (kpos, chunk, head)
        at_sb = wpool.tile([P, 2, S], BF16)   # normalized attn^T

        # ---- PE warm-up during the DMA lead-in ---------------------------
        warm = wpool.tile([P, 512], BF16)
        nc.vector.memset(warm, 1.0)
        wps = mmpool.tile([P, 512], F32, tag="mm")
        for i in range(NWARM):
            nc.tensor.matmul(wps, lhsT=warm[:, 0:P], rhs=warm,
                             start=True, stop=True)

        def qk_tile(p, wsb, dest, st):
            ps = mmpool.tile([P, 512], F32, tag="mm")
            for dc in range(DC):
                nc.tensor.matmul(
                    ps,
                    lhsT=wsb[:, dc, P * p:P * (p + 1)],
                    rhs=xt[:, dc, 512 * st:512 * (st + 1)],
                    start=(dc == 0),
                    stop=(dc == DC - 1),
                )
            nc.vector.tensor_copy(dest[:, p, 512 * st:512 * (st + 1)], ps)

        def c_and_m(p, st):
            # c[k] = SCALE * bq_h . k0_h[k] via block-diagonal bq operand,
            # for the 4 kpos chunks of seq-tile st.
            c_ps = mmpool.tile([P, 8], F32, tag="mm", name=f"c_ps_{p}_{st}")
            for j in range(4):
                ch = 4 * st + j
                nc.tensor.matmul(
                    c_ps[:, 2 * j:2 * j + 2],
                    lhsT=k_sb[:, p, P * ch:P * (ch + 1)],
                    rhs=bq_sb[:, p, :],
                    start=True,
                    stop=True,
                )
            for h in (0, 1):
                hh = 2 * p + h
                csl = slice(4 * st, 4 * st + 4)
                nc.scalar.activation(
                    m_sb[:, csl, hh],
                    c_ps.rearrange("p (j h) -> p j h", h=2)[:, :, h],
                    Exp,
                )
                # denominator column of V' is exp(c) itself
                nc.vector.tensor_copy(v_sb[:, csl, hh, DH], m_sb[:, csl, hh])

        def v_chunk(ch):
            ps = mmpool.tile([P, 512], F32, tag="mm")
            for dc in range(DC):
                nc.tensor.matmul(
                    ps[:, 0:FPC],
                    lhsT=xt[:, dc, P * ch:P * (ch + 1)],
                    rhs=wv_sb[:, dc, :],
                    start=(dc == 0),
                    stop=(dc == DC - 1),
                )
            nc.vector.tensor_copy(
                v_sb[:, ch, :, 0:DH],
                ps[:, 0:FPC].rearrange("p (h d) -> p h d", h=4),
            )
            scale_v(0, ch)

        def scale_v(p, ch):
            nc.vector.tensor_tensor(
                v_sb[:, ch, 2 * p:2 * p + 2, 0:DH],
                v_sb[:, ch, 2 * p:2 * p + 2, 0:DH],
                m_sb[:, ch, 2 * p:2 * p + 2, None].to_broadcast([P, 2, DH]),
                Mult,
            )

        def proj_tile(sm, nt):
            ps = mmpool.tile([P, 512], F32, tag="mm")
            for pc in range(2):
                nc.tensor.matmul(
                    ps,
                    lhsT=at_sb[:, pc, P * sm:P * (sm + 1)],
                    rhs=pj_sb[:, pc, 512 * nt:512 * (nt + 1)],
                    start=(pc == 0),
                    stop=(pc == 1),
                )
            stg = spool.tile([P, 512], BF16, tag="out", bufs=3)
            nc.vector.tensor_copy(stg, ps)
            nc.sync.dma_start(
                out[P * sm:P * (sm + 1), 512 * nt:512 * (nt + 1)], stg
            )

        def attention_unit(p, qt, fillers, tail=False):
            """One (pair, qtile) unit: 16 kpos chunks; per chunk a score
            pair (row-tiled heads), one exp over both heads, and the two
            AV accumulations. fillers: thunks emitted one per chunk."""
            qsl = slice(512 * qt, 512 * (qt + 1))
            pav = [avpool.tile([DH + 1, 512], F32, tag="av",
                               name=f"pav_{p}_{qt}_{h}")
                   for h in (0, 1)]
            for ch in range(NCH):
                if fillers:
                    fillers.pop(0)()
                st_t = stpool.tile([P, 2, 512], F32, tag="st",
                                   name=f"st_{p}_{qt}_{ch}")
                for h in (0, 1):
                    nc.tensor.matmul(
                        st_t[:, h, :],
                        lhsT=k_sb[DH * h:DH * (h + 1), p, P * ch:P * (ch + 1)],
                        rhs=q_sb[DH * h:DH * (h + 1), p, qsl],
                        start=True,
                        stop=True,
                    )
                e_t = epool.tile([P, 2, 512], BF16, tag="e",
                                 name=f"e_{p}_{qt}_{ch}")
                nc.scalar.activation(e_t, st_t, Exp)
                for h in (0, 1):
                    nc.tensor.matmul(
                        pav[h],
                        lhsT=v_sb[:, ch, 2 * p + h, :],
                        rhs=e_t[:, h, :],
                        start=(ch == 0),
                        stop=(ch == NCH - 1),
                    )
            while fillers:
                fillers.pop(0)()
            # ---- epilogue: normalize attn^T ------------------------------
            un = [spool.tile([DH + 1, 512], F32, tag="un", bufs=4,
                             name=f"un_{p}_{qt}_{h}")
                  for h in (0, 1)]
            for h in (0, 1):
                nc.vector.tensor_copy(un[h], pav[h])
            for h in (0, 1):
                if tail:
                    # Scalar engine is idle in the tail; exp(-ln(d)) there,
                    # then broadcast on the PE and multiply on DVE.
                    nl = spool.tile([1, 512], F32, tag="nl", bufs=2)
                    nc.scalar.activation(nl, un[h][DH:DH + 1, :], Ln)
                    rec = spool.tile([1, 512], F32, tag="rec", bufs=2)
                    nc.scalar.activation(rec, nl, Exp, scale=-1.0)
                    rb_ps = mmpool.tile([DH, 512], F32, tag="mm",
                                        name=f"rb_{p}_{qt}_{h}")
                    nc.tensor.matmul(rb_ps, lhsT=ones[0:1, :],
                                     rhs=rec, start=True, stop=True)
                    nc.vector.tensor_tensor(
                        at_sb[DH * h:DH * (h + 1), p, qsl],
                        un[h][0:DH, :],
                        rb_ps,
                        Mult,
                    )
                else:
                    rec = spool.tile([1, 512], F32, tag="rec", bufs=2)
                    nc.vector.reciprocal(rec, un[h][DH:DH + 1, :])
                    rb = spool.tile([DH, 512], F32, tag="rb", bufs=2)
                    nc.gpsimd.partition_broadcast(rb, rec)
                    nc.vector.tensor_tensor(
                        at_sb[DH * h:DH * (h + 1), p, qsl],
                        un[h][0:DH, :],
                        rb,
                        Mult,
                    )

        # ---- pre-attention minimum ---------------------------------------
        qk_tile(0, wq_sb, q_sb, 0)
        qk_tile(0, wk_sb, k_sb, 0)
        c_and_m(0, 0)
        for ch in range(4):
            v_chunk(ch)

        # ---- unit schedule with woven fillers ----------------------------
        attention_unit(0, 0, [
            lambda: qk_tile(0, wk_sb, k_sb, 1),
            lambda: (c_and_m(0, 1), v_chunk(4)),
            lambda: v_chunk(5),
            lambda: v_chunk(6),
            lambda: v_chunk(7),
            lambda: qk_tile(0, wk_sb, k_sb, 2),
            lambda: (c_and_m(0, 2), v_chunk(8)),
            lambda: v_chunk(9),
            lambda: v_chunk(10),
            lambda: qk_tile(0, wk_sb, k_sb, 3),
            lambda: (c_and_m(0, 3), v_chunk(11)),
            lambda: v_chunk(12),
            lambda: v_chunk(13),
            lambda: v_chunk(14),
            lambda: v_chunk(15),
            lambda: qk_tile(0, wq_sb, q_sb, 1),
        ])
        attention_unit(0, 1, [
            lambda: qk_tile(0, wq_sb, q_sb, 2),
            lambda: qk_tile(1, wk_sb, k_sb, 0),
            lambda: c_and_m(1, 0),
            lambda: qk_tile(1, wk_sb, k_sb, 1),
            lambda: c_and_m(1, 1),
        ])
        attention_unit(0, 2, [
            lambda: qk_tile(0, wq_sb, q_sb, 3),
            lambda: qk_tile(1, wk_sb, k_sb, 2),
            lambda: c_and_m(1, 2),
            lambda: qk_tile(1, wk_sb, k_sb, 3),
            lambda: c_and_m(1, 3),
        ] + [(lambda c0=c: scale_v(1, c0)) for c in range(8)])
        attention_unit(0, 3, [
            lambda: qk_tile(1, wq_sb, q_sb, 0),
            lambda: qk_tile(1, wq_sb, q_sb, 1),
        ] + [(lambda c0=c: scale_v(1, c0)) for c in range(8, NCH)])
        attention_unit(1, 0, [
            lambda: qk_tile(1, wq_sb, q_sb, 2),
        ])
        attention_unit(1, 1, [
            lambda: qk_tile(1, wq_sb, q_sb, 3),
        ] + [(lambda: None) for _ in range(7)] + [
            (lambda s=sm, n=nt: proj_tile(s, n))
            for sm in range(0, 4) for nt in range(2)
        ])
        attention_unit(1, 2, [(lambda: None) for _ in range(4)] + [
            (lambda s=sm, n=nt: proj_tile(s, n))
            for sm in range(4, 8) for nt in range(2)
        ])
        attention_unit(1, 3, [(lambda: None) for _ in range(4)] + [
            (lambda s=sm, n=nt: proj_tile(s, n))
            for sm in range(8, 12) for nt in range(2)
        ], tail=True)
        for sm in range(12, 16):
            for nt in range(2):
                proj_tile(sm, nt)


# ----------------------------------------------------------------------------
# host-side wrapper
# ----------------------------------------------------------------------------

_NC_CACHE = {}


def _get_nc():
    if "nc" not in _NC_CACHE:
        _NC_CACHE["nc"] = build_attention_bass()
    return _NC_CACHE["nc"]


def make_in_maps(x, qkv_w, qkv_b, proj_w):
    """Build the 8 per-core input dicts (host-side sharding)."""
    import ml_dtypes

    bf16 = ml_dtypes.bfloat16
    in_maps = []
    for c in range(8):
        b, g = divmod(c, 4)
        fsl = slice(g * FPC, (g + 1) * FPC)
        wq = (SCALE * qkv_w[0 * DIM:1 * DIM][fsl]).T     # (1024, 256)
        wk = qkv_w[1 * DIM:2 * DIM][fsl].T
        wv = qkv_w[2 * DIM:3 * DIM][fsl].T
        bq = SCALE * qkv_b[0 * DIM:1 * DIM][fsl]         # (256,)
        bqz = np.zeros((P, 2, 2), np.float32)
        for p in range(2):
            for h in range(2):
                bqz[DH * h:DH * (h + 1), p, h] = bq[(2 * p + h) * DH:(2 * p + h + 1) * DH]
        pj = proj_w[:, fsl].T                            # (256, 1024)
        in_maps.append({
            "xT": np.ascontiguousarray(x[b].T).astype(bf16),
            "wqT": np.ascontiguousarray(wq).astype(bf16),
            "wkT": np.ascontiguousarray(wk).astype(bf16),
            "wvT": np.ascontiguousarray(wv).astype(bf16),
            "bqz": bqz.astype(bf16),
            "pjT": np.ascontiguousarray(pj).astype(bf16),
        })
    return in_maps


def combine_outputs(results, qkv_b, proj_w, proj_b):
    """Sum per-pair partials and add the host-folded biases."""
    bv = qkv_b[2 * DIM:3 * DIM]
    host_bias = bv @ proj_w.T + proj_b                   # (1024,)
    out = np.empty((2, S, DIM), np.float32)
    for b in range(2):
        acc = np.zeros((S, DIM), np.float32)
        for g in range(4):
            acc += np.asarray(results[4 * b + g]["out"], np.float32)
        out[b] = acc + host_bias[None, :]
    return out


def kernel(x, qkv_w, qkv_b, proj_w, proj_b):
    x = np.asarray(x, np.float32)
    qkv_w = np.asarray(qkv_w, np.float32)
    qkv_b = np.asarray(qkv_b, np.float32)
    proj_w = np.asarray(proj_w, np.float32)
    proj_b = np.asarray(proj_b, np.float32)

    nc = _get_nc()
    in_maps = make_in_maps(x, qkv_w, qkv_b, proj_w)
    res = bass_utils.run_bass_kernel_spmd(nc, in_maps, core_ids=list(range(8)))
    return combine_outputs(res.results, qkv_b, proj_w, proj_b)


# revision 15
# speedup vs baseline: 1.2675x; 1.0247x over previous
"""MultiHeadAttention Trainium2 kernel (8-core SPMD), v3.3.

Problem: B=2, S=2048, DIM=1024, 16 heads, head_dim=64, fp32.
Sharding: core c -> (batch b = c//4, head-group g = c%4, 4 heads each).
Each core computes, for its batch and 4 heads (2 pairs of 2 heads):
    q = x Wq'^T            (Wq' = SCALE*Wq, no bias -- see bias algebra)
    k = x Wk^T             (no bias)
    v = x Wv^T             (no bias)
    S^T[k,q] = k . q       (feature-major, both heads of a pair per chunk)
    P^T = exp(S^T) scaled per-k by m[k] = exp(SCALE * bq . k[k])
    outT[d,q] = sum_k V'[k,d] P^T[k,q]   with V' = diag(m) [V | 1]
    attn^T = outT[0:64] / outT[64]       (per-q softmax denominator)
    partial[p] = attn_pair_p^T . Pj_p^T  (per-PAIR [seq,1024] projection
                                          partial, p in {0,1})
Host sums the 8 per-pair partials per batch (2 pairs x 4 cores) and adds
bv @ proj_w.T + proj_b (V-bias and proj-bias commute through softmax/proj).

Performance structure (from perfetto/HAM analysis of prior runs):
  - The softmax exp (16.8M elements/core, Scalar-engine-only at 1
    elem/cycle/lane) is the hard floor (~142us at 1024-col activations).
    Everything else is scheduled so the Scalar engine never starves.
  - Per-chunk pipeline: score pair (2 heads as adjacent matmuls with
    auto row tile_position 0/64 that co-execute in the PE array) -> one
    exp instruction covering both heads [128, 2, 512] -> 2 AV
    accumulations.
  - Per-PAIR output projection partials unlock projection matmuls as
    dense full-array filler work through the whole second half of the
    schedule, keeping the PE HAM activity monitor from throttling the
    clock to K=4/8 (the original baseline lost ~50us to that).
  - Softmax reciprocal via DVE (iterative op, [1,512]); the attn
    normalize multiply runs on the otherwise-idle GpSimd engine.  The
    last unit instead uses exp(-ln(den)) on the (then idle) Scalar
    engine and DVE multiply to shorten the kernel tail.

Bias algebra: softmax over k of SCALE*(q0+bq).(k0+bk) equals softmax of
(SCALE*q0).k0 + SCALE*bq.k0[k] -- the q0.bk and bq.bk terms are constant in k
and drop out. The per-k term is applied multiplicatively (m[k]) by scaling V
rows, and V's bias bv adds exactly bv to every attention output row.
"""

import numpy as np

import concourse.bass as bass
import concourse.mybir as mybir
import concourse.tile as tile
from concourse import bacc
from concourse import bass_utils

F32 = mybir.dt.float32
BF16 = mybir.dt.bfloat16

P = 128
DIM = 1024
S = 2048
NH = 16
DH = 64
SCALE = 1.0 / 8.0
DC = DIM // P           # 8 contraction chunks for the qkv projections
NST = S // 512          # 4 seq tiles of 512
NCH = S // P            # 16 kpos chunks of 128
FPC = 256               # features per core (4 heads * 64)
NWARM = 16              # PE warm-up matmuls


def build_attention_bass():
    nc = bacc.Bacc(
        "TRN2",
        target_bir_lowering=False,
        debug=False,
        enable_asserts=False,
        num_devices=8,
    )
    xT = nc.dram_tensor("xT", [NST, P, DC, 512], BF16, kind="ExternalInput").ap()
    wqT = nc.dram_tensor("wqT", [P, DC, FPC], BF16, kind="ExternalInput").ap()
    wkT = nc.dram_tensor("wkT", [P, DC, FPC], BF16, kind="ExternalInput").ap()
    wvT = nc.dram_tensor("wvT", [P, DC, FPC], BF16, kind="ExternalInput").ap()
    bqz = nc.dram_tensor("bqz", [P, 2, 2], BF16, kind="ExternalInput").ap()
    pjT = nc.dram_tensor("pjT", [P, 2, DIM], BF16, kind="ExternalInput").ap()
    out = nc.dram_tensor("out", [S, DIM], BF16, kind="ExternalOutput").ap()

    with tile.TileContext(nc) as tc:
        _attention_body(tc, xT, wqT, wkT, wvT, bqz, pjT, out)
    nc.compile()
    return nc


def _attention_body(tc, xT, wqT, wkT, wvT, bqz, pjT, out):
    nc = tc.nc
    Exp = mybir.ActivationFunctionType.Exp
    Ln = mybir.ActivationFunctionType.Ln
    Mult = mybir.AluOpType.mult

    with (
        tc.tile_pool(name="const", bufs=1) as cpool,
        tc.tile_pool(name="work", bufs=1) as wpool,
        tc.tile_pool(name="exp", bufs=4) as epool,
        tc.tile_pool(name="stage", bufs=3) as spool,
        tc.tile_pool(name="st", bufs=2, space="PSUM") as stpool,
        tc.tile_pool(name="mm", bufs=2, space="PSUM") as mmpool,
        tc.tile_pool(name="av", bufs=2, space="PSUM") as avpool,
    ):
        # ---- input loads (order = availability priority) -----------------
        wq_sb = cpool.tile([P, DC, FPC], BF16)
        nc.sync.dma_start(wq_sb, wqT)
        xt = cpool.tile([P, NST, DC, 512], BF16)   # st-major x
        nc.sync.dma_start(xt[:, 0], xT[0])
        wk_sb = cpool.tile([P, DC, FPC], BF16)
        nc.sync.dma_start(wk_sb, wkT)
        for st in range(1, NST):
            nc.sync.dma_start(xt[:, st], xT[st])
        wv_sb = cpool.tile([P, DC, FPC], BF16)
        nc.sync.dma_start(wv_sb, wvT)
        bq_sb = cpool.tile([P, 2, 2], BF16)
        nc.sync.dma_start(bq_sb, bqz)
        pj_sb = cpool.tile([P, 2, DIM], BF16)
        nc.sync.dma_start(pj_sb, pjT)

        ones = wpool.tile([DH + 1, DH], F32)  # bcast operand (row 64 used)
        nc.vector.memset(ones, 1.0)
        q_sb = wpool.tile([P, 2, S], BF16)    # [dh-in-pair, pair, seq]
        k_sb = wpool.tile([P, 2, S], BF16)
        v_sb = wpool.tile([P, NCH, 4, DH + 1], BF16)
        m_sb = wpool.tile([P, NCH, 4], F32)   # exp(c) per (kpos, chunk, head)
        at_sb = wpool.tile([P, 2, S], BF16)   # normalized attn^T

        # ---- PE warm-up during the DMA lead-in ---------------------------
        warm = wpool.tile([P, 512], BF16)
        nc.vector.memset(warm, 1.0)
        wps = mmpool.tile([P, 512], F32, tag="mm")
        for i in range(NWARM):
            nc.tensor.matmul(wps, lhsT=warm[:, 0:P], rhs=warm,
                             start=True, stop=True)

        def qk_tile(p, wsb, dest, st):
            ps = mmpool.tile([P, 512], F32, tag="mm")
            for dc in range(DC):
                nc.tensor.matmul(
                    ps,
                    lhsT=wsb[:, dc, P * p:P * (p + 1)],
                    rhs=xt[:, st, dc, :],
                    start=(dc == 0),
                    stop=(dc == DC - 1),
                )
            nc.vector.tensor_copy(dest[:, p, 512 * st:512 * (st + 1)], ps)

        def c_and_m(p, st):
            # c[k] = SCALE * bq_h . k0_h[k] via block-diagonal bq operand,
            # for the 4 kpos chunks of seq-tile st.
            c_ps = mmpool.tile([P, 8], F32, tag="mm", name=f"c_ps_{p}_{st}")
            for j in range(4):
                ch = 4 * st + j
                nc.tensor.matmul(
                    c_ps[:, 2 * j:2 * j + 2],
                    lhsT=k_sb[:, p, P * ch:P * (ch + 1)],
                    rhs=bq_sb[:, p, :],
                    start=True,
                    stop=True,
                )
            for h in (0, 1):
                hh = 2 * p + h
                csl = slice(4 * st, 4 * st + 4)
                nc.scalar.activation(
                    m_sb[:, csl, hh],
                    c_ps.rearrange("p (j h) -> p j h", h=2)[:, :, h],
                    Exp,
                )
                # denominator column of V' is exp(c) itself
                nc.vector.tensor_copy(v_sb[:, csl, hh, DH], m_sb[:, csl, hh])

        def v_chunk(ch):
            ps = mmpool.tile([P, 512], F32, tag="mm")
            for dc in range(DC):
                nc.tensor.matmul(
                    ps[:, 0:FPC],
                    lhsT=xt[:, ch // 4, dc, P * (ch % 4):P * (ch % 4 + 1)],
                    rhs=wv_sb[:, dc, :],
                    start=(dc == 0),
                    stop=(dc == DC - 1),
                )
            nc.vector.tensor_copy(
                v_sb[:, ch, :, 0:DH],
                ps[:, 0:FPC].rearrange("p (h d) -> p h d", h=4),
            )
            scale_v(0, ch)

        def scale_v(p, ch):
            nc.vector.tensor_tensor(
                v_sb[:, ch, 2 * p:2 * p + 2, 0:DH],
                v_sb[:, ch, 2 * p:2 * p + 2, 0:DH],
                m_sb[:, ch, 2 * p:2 * p + 2, None].to_broadcast([P, 2, DH]),
                Mult,
            )

        def proj_tile(sm, nt):
            ps = mmpool.tile([P, 512], F32, tag="mm")
            for pc in range(2):
                nc.tensor.matmul(
                    ps,
                    lhsT=at_sb[:, pc, P * sm:P * (sm + 1)],
                    rhs=pj_sb[:, pc, 512 * nt:512 * (nt + 1)],
                    start=(pc == 0),
                    stop=(pc == 1),
                )
            stg = spool.tile([P, 512], BF16, tag="out", bufs=3)
            nc.vector.tensor_copy(stg, ps)
            nc.sync.dma_start(
                out[P * sm:P * (sm + 1), 512 * nt:512 * (nt + 1)], stg
            )

        def attention_unit(p, qt, fillers, tail=False):
            """One (pair, qtile) unit: 16 kpos chunks; per chunk a score
            pair (row-tiled heads), one exp over both heads, and the two
            AV accumulations. fillers: thunks emitted one per chunk."""
            qsl = slice(512 * qt, 512 * (qt + 1))
            pav = [avpool.tile([DH + 1, 512], F32, tag="av",
                               name=f"pav_{p}_{qt}_{h}")
                   for h in (0, 1)]
            for ch in range(NCH):
                if fillers:
                    fillers.pop(0)()
                st_t = stpool.tile([P, 2, 512], F32, tag="st",
                                   name=f"st_{p}_{qt}_{ch}")
                for h in (0, 1):
                    nc.tensor.matmul(
                        st_t[:, h, :],
                        lhsT=k_sb[DH * h:DH * (h + 1), p, P * ch:P * (ch + 1)],
                        rhs=q_sb[DH * h:DH * (h + 1), p, qsl],
                        start=True,
                        stop=True,
                    )
                e_t = epool.tile([P, 2, 512], BF16, tag="e",
                                 name=f"e_{p}_{qt}_{ch}")
                nc.scalar.activation(e_t, st_t, Exp)
                for h in (0, 1):
                    nc.tensor.matmul(
                        pav[h],
                        lhsT=v_sb[:, ch, 2 * p + h, :],
                        rhs=e_t[:, h, :],
                        start=(ch == 0),
                        stop=(ch == NCH - 1),
                    )
            while fillers:
                fillers.pop(0)()
            # ---- epilogue: normalize attn^T ------------------------------
            un = [spool.tile([DH + 1, 512], F32, tag="un", bufs=4,
                             name=f"un_{p}_{qt}_{h}")
                  for h in (0, 1)]
            for h in (0, 1):
                nc.vector.tensor_copy(un[h], pav[h])

            def normalize(h):
                if tail:
                    # Scalar engine is idle in the tail; exp(-ln(d)) there,
                    # then broadcast on the PE and multiply on DVE.
                    nl = spool.tile([1, 512], F32, tag="nl", bufs=2)
                    nc.scalar.activation(nl, un[h][DH:DH + 1, :], Ln)
                    rec = spool.tile([1, 512], F32, tag="rec", bufs=2)
                    nc.scalar.activation(rec, nl, Exp, scale=-1.0)
                    rb_ps = mmpool.tile([DH, 512], F32, tag="mm",
                                        name=f"rb_{p}_{qt}_{h}")
                    nc.tensor.matmul(rb_ps, lhsT=ones[0:1, :],
                                     rhs=rec, start=True, stop=True)
                    nc.vector.tensor_tensor(
                        at_sb[DH * h:DH * (h + 1), p, qsl],
                        un[h][0:DH, :],
                        rb_ps,
                        Mult,
                    )
                else:
                    rec = spool.tile([1, 512], F32, tag="rec", bufs=2)
                    nc.vector.reciprocal(rec, un[h][DH:DH + 1, :])
                    rb = spool.tile([DH, 512], F32, tag="rb", bufs=2)
                    nc.gpsimd.partition_broadcast(rb, rec)
                    nc.vector.tensor_tensor(
                        at_sb[DH * h:DH * (h + 1), p, qsl],
                        un[h][0:DH, :],
                        rb,
                        Mult,
                    )

            if tail:
                normalize(0)
                normalize(1)
                return []
            # defer the recip chain into the next unit's filler slots so
            # the DVE backlog never blocks the boundary copies the PE
            # queue depends on
            return [lambda: normalize(0), lambda: normalize(1)]

        # ---- pre-attention minimum ---------------------------------------
        qk_tile(0, wq_sb, q_sb, 0)
        qk_tile(0, wk_sb, k_sb, 0)
        c_and_m(0, 0)
        for ch in range(4):
            v_chunk(ch)

        # ---- unit schedule with woven fillers ----------------------------
        nop = lambda: None
        epi = attention_unit(0, 0, [
            lambda: qk_tile(0, wk_sb, k_sb, 1),
            lambda: (c_and_m(0, 1), v_chunk(4)),
            lambda: v_chunk(5),
            lambda: v_chunk(6),
            lambda: v_chunk(7),
            lambda: qk_tile(0, wk_sb, k_sb, 2),
            lambda: (c_and_m(0, 2), v_chunk(8)),
            lambda: v_chunk(9),
            lambda: v_chunk(10),
            lambda: qk_tile(0, wk_sb, k_sb, 3),
            lambda: (c_and_m(0, 3), v_chunk(11)),
            lambda: v_chunk(12),
            lambda: v_chunk(13),
            lambda: v_chunk(14),
            lambda: v_chunk(15),
            lambda: qk_tile(0, wq_sb, q_sb, 1),
        ])
        epi = attention_unit(0, 1, [
            lambda: qk_tile(0, wq_sb, q_sb, 2),
            lambda: qk_tile(1, wk_sb, k_sb, 0),
            lambda: c_and_m(1, 0),
            lambda: qk_tile(1, wk_sb, k_sb, 1),
            lambda: c_and_m(1, 1),
        ] + epi)
        epi = attention_unit(0, 2, [
            lambda: qk_tile(0, wq_sb, q_sb, 3),
            lambda: qk_tile(1, wk_sb, k_sb, 2),
            lambda: c_and_m(1, 2),
            lambda: qk_tile(1, wk_sb, k_sb, 3),
            lambda: c_and_m(1, 3),
        ] + epi + [(lambda c0=c: scale_v(1, c0)) for c in range(8)])
        epi = attention_unit(0, 3, [
            lambda: qk_tile(1, wq_sb, q_sb, 0),
            lambda: qk_tile(1, wq_sb, q_sb, 1),
        ] + epi + [(lambda c0=c: scale_v(1, c0)) for c in range(8, NCH)])
        epi = attention_unit(1, 0, [
            lambda: qk_tile(1, wq_sb, q_sb, 2),
        ] + epi)
        epi = attention_unit(1, 1, [
            lambda: qk_tile(1, wq_sb, q_sb, 3),
        ] + epi + [nop, nop, nop] + [
            (lambda s=sm, n=nt: proj_tile(s, n))
            for sm in range(0, 4) for nt in range(2)
        ])
        epi = attention_unit(1, 2, epi + [nop, nop] + [
            (lambda s=sm, n=nt: proj_tile(s, n))
            for sm in range(4, 8) for nt in range(2)
        ])
        attention_unit(1, 3, epi + [nop, nop] + [
            (lambda s=sm, n=nt: proj_tile(s, n))
            for sm in range(8, 12) for nt in range(2)
        ], tail=True)
        for sm in range(12, 16):
            for nt in range(2):
                proj_tile(sm, nt)


# ----------------------------------------------------------------------------
# host-side wrapper
# ----------------------------------------------------------------------------

_NC_CACHE = {}


def _get_nc():
    if "nc" not in _NC_CACHE:
        _NC_CACHE["nc"] = build_attention_bass()
    return _NC_CACHE["nc"]


def make_in_maps(x, qkv_w, qkv_b, proj_w):
    """Build the 8 per-core input dicts (host-side sharding)."""
    import ml_dtypes

    bf16 = ml_dtypes.bfloat16
    in_maps = []
    for c in range(8):
        b, g = divmod(c, 4)
        fsl = slice(g * FPC, (g + 1) * FPC)
        wq = (SCALE * qkv_w[0 * DIM:1 * DIM][fsl]).T     # (1024, 256)
        wk = qkv_w[1 * DIM:2 * DIM][fsl].T
        wv = qkv_w[2 * DIM:3 * DIM][fsl].T
        bq = SCALE * qkv_b[0 * DIM:1 * DIM][fsl]         # (256,)
        bqz = np.zeros((P, 2, 2), np.float32)
        for p in range(2):
            for h in range(2):
                bqz[DH * h:DH * (h + 1), p, h] = bq[(2 * p + h) * DH:(2 * p + h + 1) * DH]
        pj = proj_w[:, fsl].T                            # (256, 1024)
        def wlay(w):                                 # [DIM, F] -> [P, DC, F]
            return np.ascontiguousarray(
                w.reshape(DC, P, -1).transpose(1, 0, 2)).astype(bf16)

        xr = x[b].T.reshape(DC, P, 4, 512).transpose(2, 1, 0, 3)
        in_maps.append({
            "xT": np.ascontiguousarray(xr).astype(bf16),
            "wqT": wlay(wq),
            "wkT": wlay(wk),
            "wvT": wlay(wv),
            "bqz": bqz.astype(bf16),
            "pjT": np.ascontiguousarray(
                pj.reshape(2, P, DIM).transpose(1, 0, 2)).astype(bf16),
        })
    return in_maps


def combine_outputs(results, qkv_b, proj_w, proj_b):
    """Sum per-pair partials and add the host-folded biases."""
    bv = qkv_b[2 * DIM:3 * DIM]
    host_bias = bv @ proj_w.T + proj_b                   # (1024,)
    out = np.empty((2, S, DIM), np.float32)
    for b in range(2):
        acc = np.zeros((S, DIM), np.float32)
        for g in range(4):
            acc += np.asarray(results[4 * b + g]["out"], np.float32)
        out[b] = acc + host_bias[None, :]
    return out


def kernel(x, qkv_w, qkv_b, proj_w, proj_b):
    x = np.asarray(x, np.float32)
    qkv_w = np.asarray(qkv_w, np.float32)
    qkv_b = np.asarray(qkv_b, np.float32)
    proj_w = np.asarray(proj_w, np.float32)
    proj_b = np.asarray(proj_b, np.float32)

    nc = _get_nc()
    in_maps = make_in_maps(x, qkv_w, qkv_b, proj_w)
    res = bass_utils.run_bass_kernel_spmd(nc, in_maps, core_ids=list(range(8)))
    return combine_outputs(res.results, qkv_b, proj_w, proj_b)


# revision 16
# speedup vs baseline: 1.2693x; 1.0014x over previous
"""MultiHeadAttention Trainium2 kernel (8-core SPMD), v3.3.

Problem: B=2, S=2048, DIM=1024, 16 heads, head_dim=64, fp32.
Sharding: core c -> (batch b = c//4, head-group g = c%4, 4 heads each).
Each core computes, for its batch and 4 heads (2 pairs of 2 heads):
    q = x Wq'^T            (Wq' = SCALE*Wq, no bias -- see bias algebra)
    k = x Wk^T             (no bias)
    v = x Wv^T             (no bias)
    S^T[k,q] = k . q       (feature-major, both heads of a pair per chunk)
    P^T = exp(S^T) scaled per-k by m[k] = exp(SCALE * bq . k[k])
    outT[d,q] = sum_k V'[k,d] P^T[k,q]   with V' = diag(m) [V | 1]
    attn^T = outT[0:64] / outT[64]       (per-q softmax denominator)
    partial[p] = attn_pair_p^T . Pj_p^T  (per-PAIR [seq,1024] projection
                                          partial, p in {0,1})
Host sums the 8 per-pair partials per batch (2 pairs x 4 cores) and adds
bv @ proj_w.T + proj_b (V-bias and proj-bias commute through softmax/proj).

Performance structure (from perfetto/HAM analysis of prior runs):
  - The softmax exp (16.8M elements/core, Scalar-engine-only at 1
    elem/cycle/lane) is the hard floor (~142us at 1024-col activations).
    Everything else is scheduled so the Scalar engine never starves.
  - Per-chunk pipeline: score pair (2 heads as adjacent matmuls with
    auto row tile_position 0/64 that co-execute in the PE array) -> one
    exp instruction covering both heads [128, 2, 512] -> 2 AV
    accumulations.
  - Per-PAIR output projection partials unlock projection matmuls as
    dense full-array filler work through the whole second half of the
    schedule, keeping the PE HAM activity monitor from throttling the
    clock to K=4/8 (the original baseline lost ~50us to that).
  - Softmax reciprocal via DVE (iterative op, [1,512]); the attn
    normalize multiply runs on the otherwise-idle GpSimd engine.  The
    last unit instead uses exp(-ln(den)) on the (then idle) Scalar
    engine and DVE multiply to shorten the kernel tail.

Bias algebra: softmax over k of SCALE*(q0+bq).(k0+bk) equals softmax of
(SCALE*q0).k0 + SCALE*bq.k0[k] -- the q0.bk and bq.bk terms are constant in k
and drop out. The per-k term is applied multiplicatively (m[k]) by scaling V
rows, and V's bias bv adds exactly bv to every attention output row.
"""

import numpy as np

import concourse.bass as bass
import concourse.mybir as mybir
import concourse.tile as tile
from concourse import bacc
from concourse import bass_utils

F32 = mybir.dt.float32
BF16 = mybir.dt.bfloat16

P = 128
DIM = 1024
S = 2048
NH = 16
DH = 64
SCALE = 1.0 / 8.0
DC = DIM // P           # 8 contraction chunks for the qkv projections
NST = S // 512          # 4 seq tiles of 512
NCH = S // P            # 16 kpos chunks of 128
FPC = 256               # features per core (4 heads * 64)
NWARM = 16              # PE warm-up matmuls


def build_attention_bass():
    nc = bacc.Bacc(
        "TRN2",
        target_bir_lowering=False,
        debug=False,
        enable_asserts=False,
        num_devices=8,
    )
    xT = nc.dram_tensor("xT", [NST, P, DC, 512], BF16, kind="ExternalInput").ap()
    wqT = nc.dram_tensor("wqT", [P, DC, FPC], BF16, kind="ExternalInput").ap()
    wkT = nc.dram_tensor("wkT", [P, DC, FPC], BF16, kind="ExternalInput").ap()
    wvT = nc.dram_tensor("wvT", [P, DC, FPC], BF16, kind="ExternalInput").ap()
    bqz = nc.dram_tensor("bqz", [P, 2, 2], BF16, kind="ExternalInput").ap()
    pjT = nc.dram_tensor("pjT", [P, 2, DIM], BF16, kind="ExternalInput").ap()
    out = nc.dram_tensor("out", [S, DIM], BF16, kind="ExternalOutput").ap()

    with tile.TileContext(nc) as tc:
        _attention_body(tc, xT, wqT, wkT, wvT, bqz, pjT, out)
    nc.compile()
    return nc


def _attention_body(tc, xT, wqT, wkT, wvT, bqz, pjT, out):
    nc = tc.nc
    Exp = mybir.ActivationFunctionType.Exp
    Ln = mybir.ActivationFunctionType.Ln
    Mult = mybir.AluOpType.mult

    with (
        tc.tile_pool(name="const", bufs=1) as cpool,
        tc.tile_pool(name="work", bufs=1) as wpool,
        tc.tile_pool(name="exp", bufs=4) as epool,
        tc.tile_pool(name="stage", bufs=3) as spool,
        tc.tile_pool(name="st", bufs=2, space="PSUM") as stpool,
        tc.tile_pool(name="mm", bufs=2, space="PSUM") as mmpool,
        tc.tile_pool(name="av", bufs=2, space="PSUM") as avpool,
    ):
        # ---- input loads (order = availability priority) -----------------
        wq_sb = cpool.tile([P, DC, FPC], BF16)
        nc.sync.dma_start(wq_sb, wqT)
        xt = cpool.tile([P, NST, DC, 512], BF16)   # st-major x
        nc.sync.dma_start(xt[:, 0], xT[0])
        wk_sb = cpool.tile([P, DC, FPC], BF16)
        nc.sync.dma_start(wk_sb, wkT)
        for st in range(1, NST):
            nc.sync.dma_start(xt[:, st], xT[st])
        wv_sb = cpool.tile([P, DC, FPC], BF16)
        nc.sync.dma_start(wv_sb, wvT)
        bq_sb = cpool.tile([P, 2, 2], BF16)
        nc.sync.dma_start(bq_sb, bqz)
        pj_sb = cpool.tile([P, 2, DIM], BF16)
        nc.sync.dma_start(pj_sb, pjT)

        ones = wpool.tile([DH + 1, DH], F32)  # bcast operand (row 64 used)
        nc.vector.memset(ones, 1.0)
        q_sb = wpool.tile([P, 2, S], BF16)    # [dh-in-pair, pair, seq]
        k_sb = wpool.tile([P, 2, S], BF16)
        v_sb = wpool.tile([P, NCH, 4, DH + 1], BF16)
        m_sb = wpool.tile([P, NCH, 4], F32)   # exp(c) per (kpos, chunk, head)
        at_sb = wpool.tile([P, 2, S], BF16)   # normalized attn^T

        # ---- PE warm-up during the DMA lead-in ---------------------------
        warm = wpool.tile([P, 512], BF16)
        nc.vector.memset(warm, 1.0)
        wps = mmpool.tile([P, 512], F32, tag="mm")
        for i in range(NWARM):
            nc.tensor.matmul(wps, lhsT=warm[:, 0:P], rhs=warm,
                             start=True, stop=True)

        def qk_tile(p, wsb, dest, st):
            ps = mmpool.tile([P, 512], F32, tag="mm")
            for dc in range(DC):
                nc.tensor.matmul(
                    ps,
                    lhsT=wsb[:, dc, P * p:P * (p + 1)],
                    rhs=xt[:, st, dc, :],
                    start=(dc == 0),
                    stop=(dc == DC - 1),
                )
            nc.vector.tensor_copy(dest[:, p, 512 * st:512 * (st + 1)], ps)

        def c_and_m(p, st):
            # c[k] = SCALE * bq_h . k0_h[k] via block-diagonal bq operand,
            # for the 4 kpos chunks of seq-tile st.
            c_ps = mmpool.tile([P, 8], F32, tag="mm", name=f"c_ps_{p}_{st}")
            for j in range(4):
                ch = 4 * st + j
                nc.tensor.matmul(
                    c_ps[:, 2 * j:2 * j + 2],
                    lhsT=k_sb[:, p, P * ch:P * (ch + 1)],
                    rhs=bq_sb[:, p, :],
                    start=True,
                    stop=True,
                )
            for h in (0, 1):
                hh = 2 * p + h
                csl = slice(4 * st, 4 * st + 4)
                nc.scalar.activation(
                    m_sb[:, csl, hh],
                    c_ps.rearrange("p (j h) -> p j h", h=2)[:, :, h],
                    Exp,
                )
                # denominator column of V' is exp(c) itself
                nc.vector.tensor_copy(v_sb[:, csl, hh, DH], m_sb[:, csl, hh])

        def v_chunk(ch):
            ps = mmpool.tile([P, 512], F32, tag="mm")
            for dc in range(DC):
                nc.tensor.matmul(
                    ps[:, 0:FPC],
                    lhsT=xt[:, ch // 4, dc, P * (ch % 4):P * (ch % 4 + 1)],
                    rhs=wv_sb[:, dc, :],
                    start=(dc == 0),
                    stop=(dc == DC - 1),
                )
            nc.vector.tensor_copy(
                v_sb[:, ch, :, 0:DH],
                ps[:, 0:FPC].rearrange("p (h d) -> p h d", h=4),
            )
            scale_v(0, ch)

        def scale_v(p, ch):
            nc.vector.tensor_tensor(
                v_sb[:, ch, 2 * p:2 * p + 2, 0:DH],
                v_sb[:, ch, 2 * p:2 * p + 2, 0:DH],
                m_sb[:, ch, 2 * p:2 * p + 2, None].to_broadcast([P, 2, DH]),
                Mult,
            )

        def proj_tile(sm, nt):
            ps = mmpool.tile([P, 512], F32, tag="mm")
            for pc in range(2):
                nc.tensor.matmul(
                    ps,
                    lhsT=at_sb[:, pc, P * sm:P * (sm + 1)],
                    rhs=pj_sb[:, pc, 512 * nt:512 * (nt + 1)],
                    start=(pc == 0),
                    stop=(pc == 1),
                )
            stg = spool.tile([P, 512], BF16, tag="out", bufs=3)
            nc.vector.tensor_copy(stg, ps)
            nc.sync.dma_start(
                out[P * sm:P * (sm + 1), 512 * nt:512 * (nt + 1)], stg
            )

        def attention_unit(p, qt, fillers, tail=False):
            """One (pair, qtile) unit: 16 kpos chunks; per chunk a score
            pair (row-tiled heads), one exp over both heads, and the two
            AV accumulations. fillers: thunks emitted one per chunk."""
            qsl = slice(512 * qt, 512 * (qt + 1))
            pav = [avpool.tile([DH + 1, 512], F32, tag="av",
                               name=f"pav_{p}_{qt}_{h}")
                   for h in (0, 1)]
            for ch in range(NCH):
                if fillers:
                    fillers.pop(0)()
                st_t = stpool.tile([P, 2, 512], F32, tag="st",
                                   name=f"st_{p}_{qt}_{ch}")
                for h in (0, 1):
                    nc.tensor.matmul(
                        st_t[:, h, :],
                        lhsT=k_sb[DH * h:DH * (h + 1), p, P * ch:P * (ch + 1)],
                        rhs=q_sb[DH * h:DH * (h + 1), p, qsl],
                        start=True,
                        stop=True,
                    )
                e_t = epool.tile([P, 2, 512], BF16, tag="e",
                                 name=f"e_{p}_{qt}_{ch}")
                nc.scalar.activation(e_t, st_t, Exp)
                for h in (0, 1):
                    nc.tensor.matmul(
                        pav[h],
                        lhsT=v_sb[:, ch, 2 * p + h, :],
                        rhs=e_t[:, h, :],
                        start=(ch == 0),
                        stop=(ch == NCH - 1),
                    )
            while fillers:
                fillers.pop(0)()
            # ---- epilogue: normalize attn^T ------------------------------
            un = [spool.tile([DH + 1, 512], F32, tag="un", bufs=4,
                             name=f"un_{p}_{qt}_{h}")
                  for h in (0, 1)]
            for h in (0, 1):
                nc.vector.tensor_copy(un[h], pav[h])

            def normalize(h):
                if tail:
                    # Scalar engine is idle in the tail; exp(-ln(d)) there,
                    # then broadcast on the PE and multiply on DVE.
                    nl = spool.tile([1, 512], F32, tag="nl", bufs=2)
                    nc.scalar.activation(nl, un[h][DH:DH + 1, :], Ln)
                    rec = spool.tile([1, 512], F32, tag="rec", bufs=2)
                    nc.scalar.activation(rec, nl, Exp, scale=-1.0)
                    rb_ps = mmpool.tile([DH, 512], F32, tag="mm",
                                        name=f"rb_{p}_{qt}_{h}")
                    nc.tensor.matmul(rb_ps, lhsT=ones[0:1, :],
                                     rhs=rec, start=True, stop=True)
                    nc.vector.tensor_tensor(
                        at_sb[DH * h:DH * (h + 1), p, qsl],
                        un[h][0:DH, :],
                        rb_ps,
                        Mult,
                    )
                else:
                    # The scheduler's cost model underprices RECIPROCAL
                    # 6x; deprioritize the whole chain so it never ranks
                    # ahead of boundary-critical DVE copies.
                    with tc.high_priority(-120):
                        rec = spool.tile([1, 512], F32, tag="rec", bufs=2)
                        nc.vector.reciprocal(rec, un[h][DH:DH + 1, :])
                        rb = spool.tile([DH, 512], F32, tag="rb", bufs=2)
                        nc.gpsimd.partition_broadcast(rb, rec)
                        nc.vector.tensor_tensor(
                            at_sb[DH * h:DH * (h + 1), p, qsl],
                            un[h][0:DH, :],
                            rb,
                            Mult,
                        )

            if tail:
                normalize(0)
                normalize(1)
                return []
            # defer the recip chain into the next unit's filler slots so
            # the DVE backlog never blocks the boundary copies the PE
            # queue depends on
            return [lambda: normalize(0), lambda: normalize(1)]

        # ---- pre-attention minimum ---------------------------------------
        qk_tile(0, wq_sb, q_sb, 0)
        qk_tile(0, wk_sb, k_sb, 0)
        c_and_m(0, 0)
        for ch in range(4):
            v_chunk(ch)

        # ---- unit schedule with woven fillers ----------------------------
        nop = lambda: None
        epi = attention_unit(0, 0, [
            lambda: qk_tile(0, wk_sb, k_sb, 1),
            lambda: (c_and_m(0, 1), v_chunk(4)),
            lambda: v_chunk(5),
            lambda: v_chunk(6),
            lambda: v_chunk(7),
            lambda: qk_tile(0, wk_sb, k_sb, 2),
            lambda: (c_and_m(0, 2), v_chunk(8)),
            lambda: v_chunk(9),
            lambda: v_chunk(10),
            lambda: qk_tile(0, wk_sb, k_sb, 3),
            lambda: (c_and_m(0, 3), v_chunk(11)),
            lambda: v_chunk(12),
            lambda: v_chunk(13),
            lambda: v_chunk(14),
            lambda: v_chunk(15),
            lambda: qk_tile(0, wq_sb, q_sb, 1),
        ])
        epi = attention_unit(0, 1, [
            lambda: qk_tile(0, wq_sb, q_sb, 2),
            lambda: qk_tile(1, wk_sb, k_sb, 0),
            lambda: c_and_m(1, 0),
            lambda: qk_tile(1, wk_sb, k_sb, 1),
            lambda: c_and_m(1, 1),
        ] + epi)
        epi = attention_unit(0, 2, [
            lambda: qk_tile(0, wq_sb, q_sb, 3),
            lambda: qk_tile(1, wk_sb, k_sb, 2),
            lambda: c_and_m(1, 2),
            lambda: qk_tile(1, wk_sb, k_sb, 3),
            lambda: c_and_m(1, 3),
        ] + epi + [(lambda c0=c: scale_v(1, c0)) for c in range(8)])
        epi = attention_unit(0, 3, [
            lambda: qk_tile(1, wq_sb, q_sb, 0),
            lambda: qk_tile(1, wq_sb, q_sb, 1),
        ] + epi + [(lambda c0=c: scale_v(1, c0)) for c in range(8, NCH)])
        epi = attention_unit(1, 0, [
            lambda: qk_tile(1, wq_sb, q_sb, 2),
        ] + epi)
        epi = attention_unit(1, 1, [
            lambda: qk_tile(1, wq_sb, q_sb, 3),
        ] + epi + [nop, nop, nop] + [
            (lambda s=sm, n=nt: proj_tile(s, n))
            for sm in range(0, 4) for nt in range(2)
        ])
        epi = attention_unit(1, 2, epi + [nop, nop] + [
            (lambda s=sm, n=nt: proj_tile(s, n))
            for sm in range(4, 8) for nt in range(2)
        ])
        attention_unit(1, 3, epi + [nop, nop] + [
            (lambda s=sm, n=nt: proj_tile(s, n))
            for sm in range(8, 12) for nt in range(2)
        ], tail=True)
        for sm in range(12, 16):
            for nt in range(2):
                proj_tile(sm, nt)


# ----------------------------------------------------------------------------
# host-side wrapper
# ----------------------------------------------------------------------------

_NC_CACHE = {}


def _get_nc():
    if "nc" not in _NC_CACHE:
        _NC_CACHE["nc"] = build_attention_bass()
    return _NC_CACHE["nc"]


def make_in_maps(x, qkv_w, qkv_b, proj_w):
    """Build the 8 per-core input dicts (host-side sharding)."""
    import ml_dtypes

    bf16 = ml_dtypes.bfloat16
    in_maps = []
    for c in range(8):
        b, g = divmod(c, 4)
        fsl = slice(g * FPC, (g + 1) * FPC)
        wq = (SCALE * qkv_w[0 * DIM:1 * DIM][fsl]).T     # (1024, 256)
        wk = qkv_w[1 * DIM:2 * DIM][fsl].T
        wv = qkv_w[2 * DIM:3 * DIM][fsl].T
        bq = SCALE * qkv_b[0 * DIM:1 * DIM][fsl]         # (256,)
        bqz = np.zeros((P, 2, 2), np.float32)
        for p in range(2):
            for h in range(2):
                bqz[DH * h:DH * (h + 1), p, h] = bq[(2 * p + h) * DH:(2 * p + h + 1) * DH]
        pj = proj_w[:, fsl].T                            # (256, 1024)
        def wlay(w):                                 # [DIM, F] -> [P, DC, F]
            return np.ascontiguousarray(
                w.reshape(DC, P, -1).transpose(1, 0, 2)).astype(bf16)

        xr = x[b].T.reshape(DC, P, 4, 512).transpose(2, 1, 0, 3)
        in_maps.append({
            "xT": np.ascontiguousarray(xr).astype(bf16),
            "wqT": wlay(wq),
            "wkT": wlay(wk),
            "wvT": wlay(wv),
            "bqz": bqz.astype(bf16),
            "pjT": np.ascontiguousarray(
                pj.reshape(2, P, DIM).transpose(1, 0, 2)).astype(bf16),
        })
    return in_maps


def combine_outputs(results, qkv_b, proj_w, proj_b):
    """Sum per-pair partials and add the host-folded biases."""
    bv = qkv_b[2 * DIM:3 * DIM]
    host_bias = bv @ proj_w.T + proj_b                   # (1024,)
    out = np.empty((2, S, DIM), np.float32)
    for b in range(2):
        acc = np.zeros((S, DIM), np.float32)
        for g in range(4):
            acc += np.asarray(results[4 * b + g]["out"], np.float32)
        out[b] = acc + host_bias[None, :]
    return out


def kernel(x, qkv_w, qkv_b, proj_w, proj_b):
    x = np.asarray(x, np.float32)
    qkv_w = np.asarray(qkv_w, np.float32)
    qkv_b = np.asarray(qkv_b, np.float32)
    proj_w = np.asarray(proj_w, np.float32)
    proj_b = np.asarray(proj_b, np.float32)

    nc = _get_nc()
    in_maps = make_in_maps(x, qkv_w, qkv_b, proj_w)
    res = bass_utils.run_bass_kernel_spmd(nc, in_maps, core_ids=list(range(8)))
    return combine_outputs(res.results, qkv_b, proj_w, proj_b)
